# revision 3
# baseline (speedup 1.0000x reference)
"""STICKY KV-cache layer-wise eviction kernel for Trainium2 (8 NeuronCores).

Reference computation (per head, fully head-parallel):
  1. window scores = attn[0,h,:,SINK:SINK+W*OMEGA].sum(queries) window-summed
  2. the W//4 lowest-scored windows are "losers" (jax.lax.top_k order)
  3. gather their K/V tokens, per-(head,window) asymmetric 8-bit quantize,
     dequantize -> output [2, H, Q, OMEGA, D]

Mapping on one core (4 heads):
  - scores: stream [128 q, 4064 k] chunks; PE matmuls with one-hot stationary
    matrices accumulate column sums into one PSUM bank laid out [8, 512];
    a DVE windowed reduce + SBUF->SBUF DMA flatten gives scores as a row.
  - top-k: rank[i] = #{j: s_j < s_i} + #{j<i: s_j == s_i} computed with two
    tiny PE broadcast matmuls + DVE compares; rank<Q gives the output slot
    directly, so a one-hot select matrix [W, Q] replaces the gather.
  - quantize: K/V loaded as [W windows, OMEGA*D] (each window is 4096
    contiguous floats), per-window min/max are free-axis reduces, the whole
    quantize-dequantize chain is 4 fused in-place tensor_scalar ops
    (round-half-even via the +-1.5*2^23 magic trick, matching jnp.round).
  - select: out[slot] = sel.T @ deq via PE, PSUM->SBUF->DRAM.
"""

from contextlib import ExitStack

import numpy as np

import concourse.bass as bass
import concourse.mybir as mybir
from concourse.tile import TileContext
from concourse.vector_clock import ScopedClock

F32 = mybir.dt.float32
ALU = mybir.AluOpType
AX = mybir.AxisListType

N_CORES = 8
B, H, S, D = 1, 32, 4096, 128
OMEGA, SINK = 32, 4
QMAX = 255.0
W = (S - SINK) // OMEGA          # 127 windows
Q = W // 4                       # 31 evicted windows
HPC = H // N_CORES               # 4 heads per core
KEYS = W * OMEGA                 # 4064 scored key tokens
NQ = S // 128                    # 32 query chunks per head
NSL = (KEYS + 511) // 512        # 8 key slices of <=512
WD = OMEGA * D                   # 4096 values per window
MAGIC = 12582912.0               # 1.5 * 2**23: forces round-to-nearest-even

# ---------------------------------------------------------------------------
# Workarounds for the walrus build in this container: each instruction may
# carry at most 2 sync commands (waits + updates) total.
# ---------------------------------------------------------------------------


def _patched_drain_and_barrier(self, tick_clock, wait_clock):
    nc = self.nc
    drain_inst = nc.sync.drain()
    wait_clock.add_sem_waits(
        drain_inst.ins, ScopedClock({None: tick_clock.global_clock})
    )
    si = drain_inst.ins.sync_info
    if si is not None and si.on_wait is not None and len(si.on_wait) > 1:
        waits = list(si.on_wait)
        si.on_wait = waits[:1]
        for i in range(1, len(waits)):
            extra = nc.sync.drain()
            esi = extra.ins.sync_info
            if esi is None:
                extra.ins.sync_info = mybir.SyncInfo(
                    on_wait=waits[i : i + 1], on_update=[]
                )
            else:
                esi.on_wait = waits[i : i + 1]
    nc.all_engine_barrier()
    assert self.sems is not None
    popped = nc._tile_sem_poison_stack.pop()
    assert popped is self._sem_poison
    nc.clear_and_free_semaphores(list(self.sems.allocated().values()))
    nc.all_engine_barrier()


TileContext._drain_and_barrier = _patched_drain_and_barrier


def fix_sync_overflow(nc):
    """Move excess sem-waits (beyond 2 sync commands/instruction) onto
    same-engine Drains inserted right before the oversubscribed instruction."""
    n_fix = 0
    for f in nc.m.functions:
        for bb in f.blocks:
            insts = bb.instructions
            out = []
            changed = False
            for inst in insts:
                si = inst.sync_info
                nw = len(si.on_wait) if si is not None and si.on_wait else 0
                nu = len(si.on_update) if si is not None and si.on_update else 0
                if nw + nu > 2:
                    waits = list(si.on_wait)
                    budget = max(0, 2 - nu)
                    si.on_wait = waits[:budget]
                    excess = waits[budget:]
                    while excess:
                        chunk, excess = excess[:1], excess[1:]
                        d = mybir.InstDrain(
                            name=f"I-syncfix-{n_fix}", ins=[], outs=[]
                        )
                        d.engine = inst.engine
                        d.sync_info = mybir.SyncInfo(on_wait=chunk, on_update=[])
                        out.append(d)
                        n_fix += 1
                        changed = True
                out.append(inst)
            if changed:
                bb.instructions = out
    return n_fix


# ---------------------------------------------------------------------------
# Kernel build
# ---------------------------------------------------------------------------


def build_kernel():
    nc = bass.Bass("TRN2")
    attn = nc.dram_tensor("attn", [HPC, S, S], F32, kind="ExternalInput")
    pk = nc.dram_tensor("pk", [HPC, S, D], F32, kind="ExternalInput")
    pv = nc.dram_tensor("pv", [HPC, S, D], F32, kind="ExternalInput")
    out_d = nc.dram_tensor("out", [2, HPC, Q, WD], F32, kind="ExternalOutput")

    with TileContext(nc) as tc, ExitStack() as ctx:
        cons = ctx.enter_context(tc.tile_pool(name="cons", bufs=1))
        attn_pool = ctx.enter_context(tc.tile_pool(name="attn", bufs=4))
        kv_pool = ctx.enter_context(tc.tile_pool(name="kv", bufs=3))
        small = ctx.enter_context(tc.tile_pool(name="small", bufs=3))
        outp = ctx.enter_context(tc.tile_pool(name="outp", bufs=3))
        ps_sc = ctx.enter_context(tc.tile_pool(name="ps_sc", bufs=2, space="PSUM"))
        ps_rank = ctx.enter_context(tc.tile_pool(name="ps_rank", bufs=1, space="PSUM"))
        ps_sel = ctx.enter_context(tc.tile_pool(name="ps_sel", bufs=2, space="PSUM"))

        # constants
        E8 = []
        for s8 in range(NSL):
            e = cons.tile([128, NSL], F32, tag=f"e8_{s8}")
            nc.vector.memset(e, 0.0)
            nc.vector.memset(e[:, s8 : s8 + 1], 1.0)
            E8.append(e)
        ones_row = cons.tile([1, W], F32)
        nc.vector.memset(ones_row, 1.0)
        one_1 = cons.tile([1, 1], F32)
        nc.vector.memset(one_1, 1.0)
        iota_f = cons.tile([W, Q], F32)
        nc.gpsimd.iota(iota_f, pattern=[[1, Q]], base=0, channel_multiplier=0,
                       allow_small_or_imprecise_dtypes=True)
        jt = cons.tile([W, W], F32)
        nc.gpsimd.iota(jt, pattern=[[1, W]], base=0, channel_multiplier=0,
                       allow_small_or_imprecise_dtypes=True)
        pc = cons.tile([W, 1], F32)
        nc.gpsimd.iota(pc, pattern=[[0, 1]], base=0, channel_multiplier=1,
                       allow_small_or_imprecise_dtypes=True)
        tril = cons.tile([W, W], F32)
        nc.vector.tensor_scalar(tril, jt, pc, None, op0=ALU.is_lt)

        for h in range(HPC):
            # ---- window scores -> PSUM [NSL, 512] (one bank)
            sc_ps = ps_sc.tile([NSL, 512], F32)
            for c in range(NQ):
                ch = attn_pool.tile([128, KEYS], F32, tag="chunk")
                nc.sync.dma_start(
                    out=ch,
                    in_=attn[h, c * 128 : (c + 1) * 128, SINK : SINK + KEYS],
                )
                for s in range(NSL):
                    k0 = 512 * s
                    k1 = min(512 * (s + 1), KEYS)
                    nc.tensor.matmul(
                        sc_ps[:, : k1 - k0],
                        lhsT=E8[s],
                        rhs=ch[:, k0:k1],
                        start=(c == 0 and s == 0),
                        stop=(c == NQ - 1 and s == NSL - 1),
                    )
            # windowed reduce [NSL,512] -> [NSL,16]; flatten to row [1,128]
            S_sb = small.tile([NSL, 16], F32)
            nc.vector.reduce_sum(
                S_sb, sc_ps.rearrange("p (g t) -> p g t", t=OMEGA), axis=AX.X
            )
            R_sb = small.tile([1, 128], F32)
            nc.sync.dma_start(out=R_sb, in_=S_sb)

            # ---- rank & one-hot selection matrix
            C_ps = ps_rank.tile([W, 1], F32, tag="c")
            nc.tensor.matmul(C_ps, lhsT=R_sb[:, :W], rhs=one_1)
            C_sb = small.tile([W, 1], F32)
            nc.scalar.copy(C_sb, C_ps)
            Rb_ps = ps_rank.tile([W, W], F32, tag="rb")
            nc.tensor.matmul(Rb_ps, lhsT=ones_row, rhs=R_sb[:, :W])
            cmp = small.tile([W, W], F32)
            rank1 = small.tile([W, 1], F32)
            nc.vector.tensor_scalar(
                cmp, Rb_ps, C_sb, 0.0, op0=ALU.is_lt, op1=ALU.add,
                accum_out=rank1,
            )
            eqm = small.tile([W, W], F32)
            rank2 = small.tile([W, 1], F32)
            nc.vector.scalar_tensor_tensor(
                eqm, Rb_ps, C_sb, tril, op0=ALU.is_equal, op1=ALU.mult,
                accum_out=rank2,
            )
            rank = small.tile([W, 1], F32)
            nc.vector.tensor_add(rank, rank1, rank2)
            sel = small.tile([W, Q], F32)
            nc.vector.tensor_scalar(sel, iota_f, rank, None, op0=ALU.is_equal)

            # ---- per-tensor quantize + select
            for t, src in ((0, pk), (1, pv)):
                X = kv_pool.tile([W, WD], F32, tag="x")
                nc.sync.dma_start(
                    out=X,
                    in_=src[h, SINK : SINK + KEYS, :].rearrange(
                        "(w t) d -> w (t d)", w=W
                    ),
                )
                mn = small.tile([W, 1], F32)
                mx = small.tile([W, 1], F32)
                nc.vector.tensor_reduce(mn, X, axis=AX.X, op=ALU.min)
                nc.vector.tensor_reduce(mx, X, axis=AX.X, op=ALU.max)
                scl = small.tile([W, 1], F32)
                nc.vector.tensor_sub(scl, mx, mn)
                nc.vector.tensor_scalar(
                    scl, scl, float(np.float32(1.0) / np.float32(QMAX)), 1e-8,
                    op0=ALU.mult, op1=ALU.add,
                )
                inv = small.tile([W, 1], F32)
                nc.vector.reciprocal(inv, scl)
                # q = clip(round((x - mn) * inv), 0, 255); deq = q*scl + mn
                nc.vector.tensor_scalar(X, X, mn, inv, op0=ALU.subtract, op1=ALU.mult)
                nc.vector.tensor_scalar(X, X, MAGIC, MAGIC, op0=ALU.add, op1=ALU.subtract)
                nc.vector.tensor_scalar(X, X, 0.0, QMAX, op0=ALU.max, op1=ALU.min)
                nc.vector.tensor_scalar(X, X, scl, mn, op0=ALU.mult, op1=ALU.add)

                ob = outp.tile([Q, WD], F32, tag="ob")
                for half in range(4):
                    sp = ps_sel.tile([Q, 1024], F32, tag="sp")
                    for qq in range(2):
                        f0 = 1024 * half + 512 * qq
                        nc.tensor.matmul(
                            sp[:, 512 * qq : 512 * (qq + 1)],
                            lhsT=sel,
                            rhs=X[:, f0 : f0 + 512],
                        )
                    nc.scalar.copy(ob[:, 1024 * half : 1024 * (half + 1)], sp)
                nc.sync.dma_start(out=out_d[t, h], in_=ob)

    fix_sync_overflow(nc)
    return nc


_NC_CACHE = None


def _get_nc():
    global _NC_CACHE
    if _NC_CACHE is None:
        _NC_CACHE = build_kernel()
    return _NC_CACHE


def make_in_maps(past_k, past_v, attn_score_cache):
    past_k = np.asarray(past_k, dtype=np.float32)
    past_v = np.asarray(past_v, dtype=np.float32)
    attn_score_cache = np.asarray(attn_score_cache, dtype=np.float32)
    in_maps = []
    for i in range(N_CORES):
        hs = slice(HPC * i, HPC * (i + 1))
        in_maps.append(
            {
                "attn": np.ascontiguousarray(attn_score_cache[0, hs]),
                "pk": np.ascontiguousarray(past_k[0, hs]),
                "pv": np.ascontiguousarray(past_v[0, hs]),
            }
        )
    return in_maps


def assemble(results):
    outs = [r["out"].reshape(2, HPC, Q, OMEGA, D) for r in results]
    return np.concatenate(outs, axis=1)


def kernel(past_k, past_v, attn_score_cache):
    from concourse.bass_utils import run_bass_kernel_spmd

    nc = _get_nc()
    in_maps = make_in_maps(past_k, past_v, attn_score_cache)
    res = run_bass_kernel_spmd(nc, in_maps, core_ids=list(range(N_CORES)))
    return assemble(res.results)
